# revision 7
# baseline (speedup 1.0000x reference)
"""Causal attention (B=4, S=2048, D=1024) on 8 TRN2 NeuronCores.

Sharding: core c -> (batch b = c//2, parity h = c%2).  Each core answers
the 1024 query rows x[b, h::2].

Score trick: scores = (x Wq^T)(x Wk^T)^T = x M x^T with M = Wq^T Wk.
Computing M (2.15 GF, identical on each core) replaces the Q projection
cost, and the scores matmul contracts directly against the resident
x^T -- so the K projection (4.3 GF/core redundant in the predecessor)
disappears entirely.  V is computed for own rows only and
pair-AllGathered -- the gather wire time (~10-35us/MB measured) hides
under the M / xM projections.

Queue discipline (the sync engine issues one DMA_DIRECT2D per ~610ns,
strictly in order, and a blocked descriptor blocks everything behind
it): all input loads go on the Activation HWDGE queue (packed to 25
large descriptors); the sync queue carries only the V stores, the
gather-dependent vb reloads (emitted after ALL V stores so a slow
gather cannot block them), and the output stores.

The parity interleave makes the causal workload identical on every core
(SPMD requires one program): local q i <-> global row 2i+h.  Keys (raw
x rows here) are kept parity-SECTIONED ([even rows; odd rows] --
matching the AllGather's rank-concatenation), so per section the causal
structure is h-independent except the diagonal 128x128 masks, passed as
inputs.

Layout: host passes x_all (parity-blocked, transposed), x_own^T packed
with Wv^T, and Wq packed with Wk row-major (M's contraction dim = rows
of both); scores are computed transposed [key, q] so softmax reduces
along the free axis, with the q-range trimmed per key block to the
causal triangle; exp is fused on the Scalar engine straight out of
PSUM; A@V needs no transposes (exp tiles are the stationary operand);
normalization by the softmax sum happens after A@V via a ones-column
rowsum matmul.  Compute bf16, f32 accumulation, bf16 output (host
upcasts; well within the tolerance).
"""

import os

import ml_dtypes
import numpy as np

import concourse.bass as bass
import concourse.tile as tile
from concourse import bacc, mybir
from concourse.bass_utils import run_bass_kernel_spmd

B, S, D = 4, 2048, 1024
H = S // 2           # query rows per core
NQB = H // 128       # local q blocks (8)
DCH = D // 128       # contraction chunks (8)
EB = D // 128        # feature blocks (8)
N_CORES = 8

F32 = mybir.dt.float32
BF16 = mybir.dt.bfloat16

LAST_EXEC_TIME_NS = None
LAST_TRACE_DIR = None


def _body(tc):
    nc = tc.nc
    xtp = nc.dram_tensor("xtp", [D, S], BF16, kind="ExternalInput").ap()
    xqv = nc.dram_tensor("xqv", [D, 2 * D], BF16, kind="ExternalInput").ap()
    wqk = nc.dram_tensor("wqk", [D, 2 * D], BF16, kind="ExternalInput").ap()
    maskp = nc.dram_tensor("maskp", [128, 256], BF16, kind="ExternalInput").ap()
    out = nc.dram_tensor("out", [H, D], BF16, kind="ExternalOutput").ap()

    from contextlib import ExitStack

    ctx = ExitStack()
    with ctx:
        const_pool = ctx.enter_context(tc.tile_pool(name="const", bufs=1))
        mask_sb = const_pool.tile([128, 256], BF16, name="mask_sb")
        nc.scalar.dma_start(out=mask_sb[:], in_=maskp[:, :])
        ones_sb = const_pool.tile([128, 1], BF16, name="ones_sb")
        nc.vector.memset(ones_sb[:], 1.0)

        dram = ctx.enter_context(tc.tile_pool(name="dram", bufs=1, space="DRAM"))
        # two half-gathers: halves the wire time per collective and lands the
        # first half of V much earlier (A@V consumes low key blocks first)
        # NB: addr_space="Shared" is rejected for 2-core replica groups
        v_own_h = [dram.tile([H // 2, D], BF16, name=f"v_own{i}") for i in range(2)]
        v_all_h = [dram.tile([H, D], BF16, name=f"v_all{i}") for i in range(2)]

        # long-lived tiles of the projection phase
        kv_pool = ctx.enter_context(tc.tile_pool(name="kvq", bufs=1))
        xts = [kv_pool.tile([128, S], BF16, name=f"xts{e}") for e in range(EB)]
        vb = [kv_pool.tile([128, D], BF16, name=f"vb{t}") for t in range(S // 128)]
        xmT = [kv_pool.tile([128, H], BF16, name=f"xmT{e}") for e in range(EB)]

        proj = ExitStack()
        w_pool = proj.enter_context(tc.tile_pool(name="wpool", bufs=1))
        pp = proj.enter_context(tc.tile_pool(name="pproj", bufs=8, space="PSUM"))
        vst_pool = proj.enter_context(tc.tile_pool(name="vst", bufs=3))

        xqv_sb = [w_pool.tile([128, 2 * D], BF16, name=f"xqv{d}") for d in range(DCH)]
        wqk_sb = [w_pool.tile([128, 2 * D], BF16, name=f"wqk{a}") for a in range(DCH)]
        m_sb = [w_pool.tile([128, D], BF16, name=f"m{d}") for d in range(DCH)]
        xq = [t[:, 0:D] for t in xqv_sb]      # x_own^T chunk
        wv_sb = [t[:, D : 2 * D] for t in xqv_sb]  # Wv^T chunk
        wq_sb = [t[:, 0:D] for t in wqk_sb]
        wk_sb = [t[:, D : 2 * D] for t in wqk_sb]

        # ---- input loads, all on the Activation HWDGE queue (sync queue is
        # reserved for stores + gather-gated loads).  Order by first use:
        # xqv (V proj) -> wqk (M) -> x_all^T (scores, ~80us in).
        for d in range(DCH):
            nc.scalar.dma_start(out=xqv_sb[d][:], in_=xqv[128 * d : 128 * (d + 1), :])
        for a in range(DCH):
            nc.scalar.dma_start(out=wqk_sb[a][:], in_=wqk[128 * a : 128 * (a + 1), :])
        for e in range(EB):
            nc.scalar.dma_start(out=xts[e][:], in_=xtp[128 * e : 128 * (e + 1), :])

        # ---- V projection for own rows -> DRAM -> two pair AllGathers.
        # Gathered half hf: rows [0:512) from pair-rank 0 (even global
        # rows), [512:1024) from rank 1 (odd) -- so half hf serves
        # vb[4*hf .. 4*hf+4) (even section) and vb[8+4*hf ..) (odd).
        for hf in range(2):
            # contraction-OUTER over all 8 PSUM banks: each arriving x chunk
            # immediately unlocks 8 matmuls, so the PE streams behind the
            # input DMA instead of stalling on the first chain's last chunk;
            # the whole half finishes together, triggering the gather early.
            pss = [
                [
                    pp.tile([128, 512], F32, tag="psp", name=f"psv{4*hf+t2}_{eh}")
                    for eh in range(2)
                ]
                for t2 in range(NQB // 2)
            ]
            for d in range(DCH):
                for t2 in range(NQB // 2):
                    t = 4 * hf + t2
                    for eh in range(2):
                        nc.tensor.matmul(
                            out=pss[t2][eh][:],
                            lhsT=xq[d][:, 128 * t : 128 * (t + 1)],
                            rhs=wv_sb[d][:, 512 * eh : 512 * (eh + 1)],
                            start=(d == 0),
                            stop=(d == DCH - 1),
                        )
            for t2 in range(NQB // 2):
                t = 4 * hf + t2
                vst = vst_pool.tile([128, D], BF16, tag="vst", name=f"vst{t}")
                for eh in range(2):
                    nc.vector.tensor_copy(
                        out=vst[:, 512 * eh : 512 * (eh + 1)], in_=pss[t2][eh][:]
                    )
                nc.sync.dma_start(
                    out=v_own_h[hf][128 * t2 : 128 * (t2 + 1), :], in_=vst[:]
                )
            nc.gpsimd.collective_compute(
                "AllGather",
                mybir.AluOpType.bypass,
                replica_groups=[[0, 1], [2, 3], [4, 5], [6, 7]],
                ins=[v_own_h[hf].opt()],
                outs=[v_all_h[hf].opt()],
            )
        # vb reloads AFTER all V stores (sync queue is in-order; a slow
        # gather must not block the second gather's input stores)
        for hf in range(2):
            for t2 in range(NQB // 2):
                for p in range(2):
                    t = 8 * p + 4 * hf + t2
                    nc.sync.dma_start(
                        out=vb[t][:],
                        in_=v_all_h[hf][
                            512 * p + 128 * t2 : 512 * p + 128 * (t2 + 1), :
                        ],
                    )

        # ---- M = Wq^T Wk  (contraction over rows of both; out [d, d'])
        for db in range(DCH):
            for dh in range(2):
                ps = pp.tile([128, 512], F32, tag="psp", name=f"psm{db}_{dh}")
                for a in range(DCH):
                    nc.tensor.matmul(
                        out=ps[:],
                        lhsT=wq_sb[a][:, 128 * db : 128 * (db + 1)],
                        rhs=wk_sb[a][:, 512 * dh : 512 * (dh + 1)],
                        start=(a == 0),
                        stop=(a == DCH - 1),
                    )
                # ACT is otherwise idle here; keep DVE free
                nc.scalar.activation(
                    out=m_sb[db][:, 512 * dh : 512 * (dh + 1)],
                    in_=ps[:],
                    func=mybir.ActivationFunctionType.Copy,
                    bias=0.0,
                    scale=1.0,
                )

        # ---- xM^T[d', q] = sum_d M[d, d'] x_own^T[d, q]
        for t in range(EB):
            for qh in range(2):
                ps = pp.tile([128, 512], F32, tag="psp", name=f"psq{t}_{qh}")
                for d in range(DCH):
                    nc.tensor.matmul(
                        out=ps[:],
                        lhsT=m_sb[d][:, 128 * t : 128 * (t + 1)],
                        rhs=xq[d][:, 512 * qh : 512 * (qh + 1)],
                        start=(d == 0),
                        stop=(d == DCH - 1),
                    )
                nc.vector.tensor_copy(out=xmT[t][:, 512 * qh : 512 * (qh + 1)], in_=ps[:])

        proj.close()  # free w/m staging SBUF + projection PSUM

        # ---- attention (keys parity-sectioned: section p holds rows p::2)
        expT_pool = ctx.enter_context(tc.tile_pool(name="expT", bufs=2))
        psc = ctx.enter_context(tc.tile_pool(name="psc", bufs=2, space="PSUM"))
        pav = ctx.enter_context(tc.tile_pool(name="pav", bufs=4, space="PSUM"))
        prs = ctx.enter_context(tc.tile_pool(name="prs", bufs=2, space="PSUM"))
        out_pool = ctx.enter_context(tc.tile_pool(name="outp", bufs=2))
        rec_pool = ctx.enter_context(tc.tile_pool(name="rec", bufs=2))

        inv_sqrt_d = float(1.0 / np.sqrt(D))
        for Sx in (1, 0):  # q supers of 512 local rows (1 first: shorter tail)
            slab = expT_pool.tile([128, 16, 512], BF16, tag="slab", name=f"slab{Sx}")
            # scoresT + exp (+ diagonal causal fixups); K is the key block
            # index within parity section p.  q-range trimmed to the causal
            # triangle: key block K only serves local q blocks J >= K.
            for p in range(2):
                for K in range(4 * (Sx + 1)):
                    qs = 128 * max(0, K - 4 * Sx)  # trim start within super
                    ps = psc.tile([128, 512], F32, tag="pssc", name=f"pss{Sx}_{p}_{K}")
                    for e in range(EB):
                        nc.tensor.matmul(
                            out=ps[:, qs:512],
                            lhsT=xts[e][:, 1024 * p + 128 * K : 1024 * p + 128 * (K + 1)],
                            rhs=xmT[e][:, 512 * Sx + qs : 512 * (Sx + 1)],
                            start=(e == 0),
                            stop=(e == EB - 1),
                        )
                    nc.scalar.activation(
                        out=slab[:, 8 * p + K, qs:512],
                        in_=ps[:, qs:512],
                        func=mybir.ActivationFunctionType.Exp,
                        bias=0.0,
                        scale=inv_sqrt_d,
                    )
                    # q block J owns the diagonal key block K == J in each section
                    if 4 * Sx <= K < 4 * (Sx + 1):
                        qo = 128 * (K - 4 * Sx)
                        nc.vector.tensor_mul(
                            out=slab[:, 8 * p + K, qo : qo + 128],
                            in0=slab[:, 8 * p + K, qo : qo + 128],
                            in1=mask_sb[:, 128 * p : 128 * (p + 1)],
                        )
            # A@V + rowsum + normalize + store per 128-row q block
            # (descending J so the last accumulation chain is the shortest)
            for Jr in reversed(range(4)):
                J = 4 * Sx + Jr
                qo = 128 * Jr
                av0 = pav.tile([128, 512], F32, tag="av", name=f"av0_{J}")
                av1 = pav.tile([128, 512], F32, tag="av", name=f"av1_{J}")
                rs = prs.tile([128, 1], F32, tag="rs", name=f"rs{J}")
                n_acc = 2 * (J + 1)
                i = 0
                for p in range(2):
                    for K in range(J + 1):
                        lw = slab[:, 8 * p + K, qo : qo + 128]
                        first, last = (i == 0), (i == n_acc - 1)
                        vt = vb[8 * p + K]
                        nc.tensor.matmul(
                            out=av0[:], lhsT=lw, rhs=vt[:, 0:512], start=first, stop=last
                        )
                        nc.tensor.matmul(
                            out=av1[:], lhsT=lw, rhs=vt[:, 512:1024], start=first, stop=last
                        )
                        nc.tensor.matmul(
                            out=rs[:], lhsT=lw, rhs=ones_sb[:], start=first, stop=last
                        )
                        i += 1
                rec = rec_pool.tile([128, 1], F32, tag="rec", name=f"rec{J}")
                nc.vector.reciprocal(out=rec[:], in_=rs[:])
                ot = out_pool.tile([128, D], BF16, tag="ot", name=f"ot{J}")
                nc.vector.tensor_scalar_mul(out=ot[:, 0:512], in0=av0[:], scalar1=rec[:])
                nc.vector.tensor_scalar_mul(out=ot[:, 512:1024], in0=av1[:], scalar1=rec[:])
                nc.sync.dma_start(out=out[128 * J : 128 * (J + 1), :], in_=ot[:])


_PROGRAM = None


def _build_program():
    global _PROGRAM
    if _PROGRAM is None:
        nc = bacc.Bacc("TRN2", target_bir_lowering=False, debug=False, num_devices=N_CORES)
        with tile.TileContext(nc) as tc:
            _body(tc)
        nc.compile()
        _PROGRAM = nc
    return _PROGRAM


def _install_ntff_hook():
    """Recreate the missing antenv.axon_hooks so trace=True can profile."""
    import sys
    import types

    if "antenv.axon_hooks" in sys.modules:
        return
    import concourse.bass_utils as bass_utils
    from trn_agent_boot.trn_boot import _ntff_profile_via_ctypes

    hook = _ntff_profile_via_ctypes("/opt/axon/libaxon_pjrt.so")
    mod = types.ModuleType("antenv.axon_hooks")
    mod._hook = hook
    mod.get_axon_ntff_profile_hook = lambda: mod._hook
    mod.set_axon_ntff_profile_hook = lambda h: None

    sys.modules["antenv.axon_hooks"] = mod
    bass_utils.upload_artifacts = lambda tmpdir: "local://" + tmpdir


def kernel(x, wq, wk, wv):
    global LAST_EXEC_TIME_NS, LAST_TRACE_DIR
    x = np.asarray(x, dtype=np.float32)
    wq = np.asarray(wq, dtype=np.float32)
    wk = np.asarray(wk, dtype=np.float32)
    wv = np.asarray(wv, dtype=np.float32)

    nc = _build_program()

    bf16 = ml_dtypes.bfloat16
    wqk = np.ascontiguousarray(
        np.concatenate([wq, wk], axis=1).astype(bf16)
    )
    wvT = wv.T.astype(bf16)
    idx = np.arange(128)
    kk, qq = np.meshgrid(idx, idx, indexing="ij")
    # even section (keys 2k vs q 2i+h): keep iff k <= i, both parities
    mask_even = (kk <= qq).astype(bf16)
    # odd section (keys 2k+1 vs q 2i+h): keep iff k <= i + h - 1
    masks_odd = [(kk <= qq + h - 1).astype(bf16) for h in range(2)]
    maskps = [
        np.ascontiguousarray(np.concatenate([mask_even, masks_odd[h]], axis=1))
        for h in range(2)
    ]

    in_maps = []
    for c in range(N_CORES):
        b, h = c // 2, c % 2
        xpb = np.concatenate([x[b, 0::2, :], x[b, 1::2, :]], axis=0)  # parity-blocked
        xqv = np.concatenate([x[b, h::2, :].T.astype(bf16), wvT], axis=1)
        in_maps.append(
            {
                "xtp": np.ascontiguousarray(xpb.T.astype(bf16)),
                "xqv": np.ascontiguousarray(xqv),
                "wqk": wqk,
                "maskp": maskps[h],
            }
        )

    profile = os.environ.get("KERNEL_PROFILE", "0") == "1"
    if profile:
        _install_ntff_hook()
        import tempfile

        tmpdir = tempfile.mkdtemp(prefix="attn_trace_")
        res = run_bass_kernel_spmd(
            nc, in_maps, core_ids=list(range(N_CORES)), trace=True, tmpdir=tmpdir
        )
        LAST_EXEC_TIME_NS = res.exec_time_ns
        LAST_TRACE_DIR = tmpdir
    else:
        res = run_bass_kernel_spmd(nc, in_maps, core_ids=list(range(N_CORES)))

    out = np.empty((B, S, D), dtype=np.float32)
    for c in range(N_CORES):
        b, h = c // 2, c % 2
        out[b, h::2, :] = res.results[c]["out"].astype(np.float32)
    return out
